# revision 28
# baseline (speedup 1.0000x reference)
"""Trainium2 Bass kernel for nn_DenseConcatAttentionScore.

Math (reference):
    Wq, Wk = W[:Dq], W[Dq:]
    score[b, t] = v . tanh(q[b] @ Wq + k[b, t] @ Wk + bias)

Sharding: data-parallel over batch B=32 across 8 NeuronCores (4 batches per
core); W/bias/v replicated. k is pre-transposed host-side so the contraction
dim D lands on SBUF partitions.

Speed trick vs the bf16 baseline: fp8 DoubleRow matmuls contract 256 rows
per pass (2 MACs/cell/cycle), so one DR matmul replaces two bf16 matmuls.
Pure e4m3 fails the 2e-2 accuracy gate (rel ~2.6e-2), so a hybrid is used:
on 3 of every 4 m-tiles, d<256 is contracted in one scaled-e4m3 DR matmul
(k*8, Wk*256 - scaling dodges e4m3's denormal floor at |x|<2^-6) and
d>=256 in bf16; every 4th m-tile is all-bf16. Measured rel err ~1.65e-2.
All bf16 W chunks are pre-scaled x2048 to match the fp8 product scale
(8*256); the tanh activation rescales by 1/2048 via its scale operand.

Device pipeline per core (M = 4*4096 = 16384 rows, m-tiles of 512):
    kp[a, m] = sum_d Wk[d, a] * kT[d, m]      (PE: 3 matmuls hybrid / 4 bf16)
    th[a, m] = tanh(kp[a, m]/2048 + qp[a, b] + bias[a])   (ACT, per-partition
                                                           bias, bf16 out)
    score[m] = sum_a v[a] * th[a, m]          (PE, 4 column-tiled 128x32
                                               matmuls run concurrently)
"""

import sys

import ml_dtypes
import numpy as np

for _p in ("/opt/trn_rl_repo",):
    if _p not in sys.path:
        sys.path.append(_p)

import concourse.bass as bass
import concourse.mybir as mybir
import concourse.tile as tile
from concourse import bass_utils

B, T, D, A = 32, 4096, 512, 512
NCORES = 8
BPC = B // NCORES            # batches per core
M = BPC * T                  # rows per core
MT_FREE = 512                # moving free dim per matmul
MT = M // MT_FREE            # m-tiles per core
P = 128
AC = A // P                  # a-chunks
DC = D // P                  # d-chunks
TPB = T // MT_FREE           # m-tiles per batch

SK = 8.0                     # fp8 k scale
SW8 = 256.0                  # fp8 W scale
SCALE = SK * SW8             # psum scale; undone by the tanh activation

F32 = mybir.dt.float32
BF16 = mybir.dt.bfloat16
FP16 = mybir.dt.float16
E4 = mybir.dt.float8e4
E4NP = ml_dtypes.float8_e4m3fn
BF16NP = ml_dtypes.bfloat16
FP16NP = np.float16


def _is_bf16_tile(i):
    # every 4th m-tile all-bf16 keeps the fp8 quantization error at
    # sqrt(3/4) of the pure-hybrid level (rel 1.65e-2 < the 2e-2 gate)
    return i % 4 == 3


def _split_excess_waits(nc, max_waits=1):
    """This walrus build's CoreV3 codegen rejects instructions carrying more
    than one sem wait (setupSyncWait: 'Too many sync wait commands'). Move
    excess waits onto NoOps inserted immediately before the offender — the
    engine executes in order, so sequential waits are equivalent."""
    ctr = 0
    for f in nc.m.functions:
        for blk in f.blocks:
            out = []
            changed = False
            for inst in blk.instructions:
                si = inst.sync_info
                nw = len(si.on_wait) if (si is not None and si.on_wait) else 0
                if nw > max_waits:
                    waits = list(si.on_wait)
                    keep, extra = waits[-max_waits:], waits[:-max_waits]
                    for i in range(0, len(extra), max_waits):
                        nop = mybir.InstNoOp(name=f"I-waitsplit-{ctr}")
                        ctr += 1
                        nop.engine = inst.engine
                        nop.sync_info = mybir.SyncInfo(
                            on_wait=extra[i:i + max_waits], on_update=[])
                        out.append(nop)
                    inst.sync_info = mybir.SyncInfo(
                        on_wait=keep, on_update=list(si.on_update or []))
                    changed = True
                out.append(inst)
            if changed:
                blk.instructions = out
    return ctr


def _build():
    nc = bass.Bass("TRN2", target_bir_lowering=False, debug=False)
    # tile-major layouts: per (partition, m-tile) the payload is contiguous
    kt8 = nc.dram_tensor("kt8", [P, MT, 2, MT_FREE], E4,
                         kind="ExternalInput").ap()
    kthi = nc.dram_tensor("kthi", [P, MT, 2, MT_FREE], FP16,
                          kind="ExternalInput").ap()
    ktlo = nc.dram_tensor("ktlo", [P, MT, 2, MT_FREE], FP16,
                          kind="ExternalInput").ap()
    qT = nc.dram_tensor("qT", [D, BPC], FP16, kind="ExternalInput").ap()
    wq = nc.dram_tensor("wq", [D, A], FP16, kind="ExternalInput").ap()
    wk8 = nc.dram_tensor("wk8", [P, 2, A], E4, kind="ExternalInput").ap()
    wkhi = nc.dram_tensor("wkhi", [P, 2, A], FP16, kind="ExternalInput").ap()
    wklo = nc.dram_tensor("wklo", [P, 2, A], FP16, kind="ExternalInput").ap()
    bias = nc.dram_tensor("bias", [A], F32, kind="ExternalInput").ap()
    vv = nc.dram_tensor("v", [A], FP16, kind="ExternalInput").ap()
    out = nc.dram_tensor("out", [MT, MT_FREE], F32, kind="ExternalOutput").ap()

    GROUP = 4                  # m-tiles per col-tiled v-dot batch
    NB = MT // GROUP

    with tile.TileContext(nc) as tc:
        with tc.tile_pool(name="singles", bufs=1) as singles, \
             tc.tile_pool(name="ktp8", bufs=8) as ktp8, \
             tc.tile_pool(name="ktph", bufs=8) as ktph, \
             tc.tile_pool(name="ktpl", bufs=3) as ktpl, \
             tc.tile_pool(name="thp", bufs=140) as thp, \
             tc.tile_pool(name="scp", bufs=3) as scp, \
             tc.tile_pool(name="psum_kp", bufs=5, space="PSUM") as psum_kp, \
             tc.tile_pool(name="psum_sm", bufs=3, space="PSUM") as psum_sm:

            # PE warm-up: a few matmuls on a zeroed SBUF tile ramp the PE
            # clock while the head DMAs are still in flight. The memset
            # leads the vector queue so warm-up starts immediately.
            warm_sb = singles.tile([P, MT_FREE], FP16, name="warm_sb")
            nc.vector.memset(warm_sb, 0.0)

            # Head DMA routing. A single HWDGE queue moves only ~140GB/s and
            # processes transfers in issue order, so the sync queue is
            # ordered by first-need: wk8 + kt8-0 (the first DR matmul's
            # operands, 2KB/partition) lead, then kthi-0/wkhi for the bf16
            # matmuls that follow ~200ns later. The qp feed rides the gpsimd
            # queue. (Scalar-queue DMAs measurably slow the whole core —
            # avoid.)
            wk8_sb = singles.tile([P, 2, A], E4, name="wk8_sb")
            nc.sync.dma_start(out=wk8_sb, in_=wk8)
            wkhi_sb = None
            wq_sb = singles.tile([P, DC, A], FP16, name="wq_sb")
            nc.gpsimd.dma_start(out=wq_sb, in_=wq.rearrange("(dc p) a -> p dc a", p=P))
            qT_sb = singles.tile([P, DC, BPC], FP16, name="qT_sb")
            nc.gpsimd.dma_start(out=qT_sb, in_=qT.rearrange("(dc p) b -> p dc b", p=P))
            bT_sb = singles.tile([P, AC], F32, name="bT_sb")
            nc.gpsimd.dma_start(out=bT_sb, in_=bias.rearrange("(ac p) -> p ac", p=P))

            kt_tiles = {}

            def get_kt(i):
                if i not in kt_tiles and i < MT:
                    if _is_bf16_tile(i):
                        lo = ktpl.tile([P, 2, MT_FREE], FP16,
                                       name=f"ktlo{i}", tag="ktl")
                        nc.sync.dma_start(out=lo, in_=ktlo[:, i])
                    else:
                        lo = ktp8.tile([P, 2, MT_FREE], E4,
                                       name=f"kt8{i}", tag="kt8")
                        nc.sync.dma_start(out=lo, in_=kt8[:, i])
                    hi = ktph.tile([P, 2, MT_FREE], FP16,
                                   name=f"kthi{i}", tag="kth")
                    nc.sync.dma_start(out=hi, in_=kthi[:, i])
                    kt_tiles[i] = (lo, hi)
                return kt_tiles.get(i)

            get_kt(0)
            wkhi_sb = singles.tile([P, 2, A], FP16, name="wkhi_sb")
            nc.sync.dma_start(out=wkhi_sb, in_=wkhi)
            get_kt(1)
            v_sb = singles.tile([P, AC], FP16, name="v_sb")
            nc.gpsimd.dma_start(out=v_sb, in_=vv.rearrange("(ac p) -> p ac", p=P))
            wklo_sb = singles.tile([P, 2, A], FP16, name="wklo_sb")
            nc.gpsimd.dma_start(out=wklo_sb, in_=wklo)
            qpb_sb = singles.tile([P, AC, BPC], F32, name="qpb_sb")
            for i in range(2, 4):
                get_kt(i)

            # PE warm-up sized to the clock ramp (~3us of continuous busy)
            # so kp starts at full clock right as kt0 lands.
            warm_ps = psum_kp.tile([P, MT_FREE], F32, name="warm_ps", tag="kp")
            for _ in range(7):
                nc.tensor.matmul(warm_ps, lhsT=warm_sb[:, :P], rhs=warm_sb,
                                 start=True, stop=True)

            def vdot_batch(bidx, th_grid):
                # score for GROUP m-tiles in one PSUM bank: strip j holds
                # m-tile j at partition 32j, accumulated over a-chunks.
                # 4 col-tiled matmuls per wave run concurrently (128x32 mode).
                score_ps = psum_sm.tile([P, MT_FREE], F32,
                                        name=f"score_ps{bidx}", tag="sm")
                for ac in range(AC):
                    for j in range(GROUP):
                        nc.tensor.matmul(score_ps[32 * j:32 * j + 1, :],
                                         lhsT=v_sb[:, ac:ac + 1],
                                         rhs=th_grid[j][ac],
                                         start=(ac == 0), stop=(ac == AC - 1),
                                         tile_position=(0, 32 * j))
                sc = scp.tile([P, MT_FREE], F32, name=f"sc{bidx}", tag="sc")
                nc.vector.tensor_copy(sc, score_ps)
                eng = nc.sync if bidx >= NB - 2 else nc.gpsimd
                eng.dma_start(out=out[bidx * GROUP:(bidx + 1) * GROUP, :],
                              in_=sc[0:P:32, :])

            def emit_kp(i, ac):
                lo, hi = kt_tiles[i]
                kp_ps = psum_kp.tile([P, MT_FREE], F32,
                                     name=f"kp{i}_{ac}", tag="kp")
                acs = slice(ac * P, (ac + 1) * P)
                if _is_bf16_tile(i):
                    for dc in range(2):
                        nc.tensor.matmul(kp_ps, lhsT=wklo_sb[:, dc, acs],
                                         rhs=lo[:, dc, :],
                                         start=(dc == 0), stop=False)
                else:
                    # one DoubleRow fp8 matmul contracts d<256 in 512 cycles
                    nc.tensor.matmul(kp_ps, lhsT=wk8_sb[:, :, acs],
                                     rhs=lo,
                                     start=True, stop=False,
                                     perf_mode=mybir.MatmulPerfMode.DoubleRow)
                for dc in range(2):
                    nc.tensor.matmul(kp_ps, lhsT=wkhi_sb[:, dc, acs],
                                     rhs=hi[:, dc, :],
                                     start=False, stop=(dc == 1))
                return kp_ps

            def emit_tanh(i, ac, kp_ps):
                th = thp.tile([P, MT_FREE], FP16, name=f"th{i}_{ac}", tag="th")
                nc.scalar.activation(out=th, in_=kp_ps,
                                     func=mybir.ActivationFunctionType.Tanh,
                                     bias=qpb_sb[:, ac, (i // TPB):(i // TPB) + 1],
                                     scale=1.0 / SCALE)
                return th

            def emit_mtile(i):
                get_kt(i)
                get_kt(i + 4)
                get_kt(i + 5)
                return [emit_tanh(i, ac, emit_kp(i, ac)) for ac in range(AC)]

            # first m-tile's kp matmuls run before qp: they only need wk +
            # kt0, which lead the sync queue. qp (fed via gpsimd) follows;
            # m0's tanh must follow qp (it reads qpb).
            kp0 = [emit_kp(0, ac) for ac in range(AC)]

            # qp[a, b] = sum_d Wq[d, a] q[b, d], then + bias[a]; [a] on partitions
            for ac in range(AC):
                qp_ps = psum_sm.tile([P, BPC], F32, name=f"qp_ps{ac}", tag="sm")
                for dc in range(DC):
                    nc.tensor.matmul(qp_ps,
                                     lhsT=wq_sb[:, dc, ac * P:(ac + 1) * P],
                                     rhs=qT_sb[:, dc, :],
                                     start=(dc == 0), stop=(dc == DC - 1))
                nc.vector.tensor_scalar_add(qpb_sb[:, ac, :], qp_ps,
                                            bT_sb[:, ac:ac + 1])

            th0 = [emit_tanh(0, ac, kp0[ac]) for ac in range(AC)]
            get_kt(4)

            pending = []
            for bi in range(NB):
                if bi == NB - 1:
                    # drain everything before the last batch's kp matmuls so
                    # only one batch's v-dot trails the final kp stream
                    while pending:
                        b0, g0 = pending.pop(0)
                        vdot_batch(b0, g0)
                th_grid = [th0] if bi == 0 else []
                for j in range(1 if bi == 0 else 0, GROUP):
                    th_grid.append(emit_mtile(bi * GROUP + j))
                pending.append((bi, th_grid))
                # drain vdots in groups of 8 batches: one 128x32-mode window
                # per group keeps PE mode switches rare
                if len(pending) >= 9:
                    while len(pending) > 1:
                        b0, g0 = pending.pop(0)
                        vdot_batch(b0, g0)
            for b0, g0 in pending:
                vdot_batch(b0, g0)

    _split_excess_waits(nc)
    return nc


_NC = None


def _get_nc():
    global _NC
    if _NC is None:
        _NC = _build()
    return _NC


def _prep_core(kc):
    """kc: [M, D] f32 for one core -> tile-major kt8/kthi/ktlo arrays."""
    kT = np.ascontiguousarray(kc.T)                      # [D, M]
    # [d, m] -> [p, mt, two, mf] with d = two*128 + p
    def tilemajor(a, dt):
        return np.ascontiguousarray(
            a.reshape(2, P, MT, MT_FREE).transpose(1, 2, 0, 3)).astype(dt)
    return (
        tilemajor(kT[:256] * SK, E4NP),
        tilemajor(kT[256:], FP16NP),
        tilemajor(kT[:256], FP16NP),
    )


def run_sharded(inputs, **run_kwargs):
    q = np.ascontiguousarray(np.asarray(inputs["q"], np.float32))
    k = np.ascontiguousarray(np.asarray(inputs["k"], np.float32))
    W = np.asarray(inputs["W"], np.float32)
    b = np.ascontiguousarray(np.asarray(inputs["b"], np.float32))
    v = np.ascontiguousarray(np.asarray(inputs["v"], np.float32))
    nc = _get_nc()
    wq = np.ascontiguousarray(W[:D])
    wk = np.ascontiguousarray(W[D:])
    wk8 = np.ascontiguousarray(
        (wk[:256] * SW8).reshape(2, P, A).transpose(1, 0, 2)).astype(E4NP)
    wkhi = np.ascontiguousarray(
        (wk[256:] * SCALE).reshape(2, P, A).transpose(1, 0, 2)).astype(FP16NP)
    wklo = np.ascontiguousarray(
        (wk[:256] * SCALE).reshape(2, P, A).transpose(1, 0, 2)).astype(FP16NP)
    in_maps = []
    for c in range(NCORES):
        kc = k[c * BPC:(c + 1) * BPC].reshape(M, D)
        kt8, kthi, ktlo = _prep_core(kc)
        in_maps.append({
            "kt8": kt8, "kthi": kthi, "ktlo": ktlo,
            "qT": np.ascontiguousarray(q[c * BPC:(c + 1) * BPC].T).astype(FP16NP),
            "wq": wq.astype(FP16NP), "wk8": wk8, "wkhi": wkhi, "wklo": wklo,
            "bias": b, "v": v.astype(FP16NP),
        })
    return bass_utils.run_bass_kernel_spmd(nc, in_maps, list(range(NCORES)),
                                           **run_kwargs)


def kernel(q, k, W, b, v):
    res = run_sharded({"q": q, "k": k, "W": W, "b": b, "v": v})
    outs = [res.results[c]["out"].reshape(BPC, T) for c in range(NCORES)]
    return np.concatenate(outs, axis=0)


if __name__ == "__main__":
    rng = np.random.default_rng(0)
    ins = {
        "q": rng.standard_normal((B, D), dtype=np.float32),
        "k": rng.standard_normal((B, T, D), dtype=np.float32),
        "W": (rng.standard_normal((2 * D, A)) * 0.02).astype(np.float32),
        "b": np.zeros((A,), np.float32),
        "v": (rng.standard_normal((A,)) * (2.0 / A) ** 0.5).astype(np.float32),
    }
    got = kernel(**ins)
    Wq, Wk = ins["W"][:D], ins["W"][D:]
    qp = ins["q"] @ Wq
    kp = ins["k"] @ Wk
    ref = np.tanh(qp[:, None, :] + kp + ins["b"]).astype(np.float32) @ ins["v"]
    err = np.abs(got - ref)
    rel = np.linalg.norm(got - ref) / np.linalg.norm(ref)
    print("max abs err:", err.max(), "rel:", rel)


# revision 29
# speedup vs baseline: 1.0193x; 1.0193x over previous
"""Trainium2 Bass kernel for nn_DenseConcatAttentionScore.

Math (reference):
    Wq, Wk = W[:Dq], W[Dq:]
    score[b, t] = v . tanh(q[b] @ Wq + k[b, t] @ Wk + bias)

Sharding: data-parallel over batch B=32 across 8 NeuronCores (4 batches per
core); W/bias/v replicated. k is pre-transposed host-side so the contraction
dim D lands on SBUF partitions.

Speed trick vs the bf16 baseline: fp8 DoubleRow matmuls contract 256 rows
per pass (2 MACs/cell/cycle), so one DR matmul replaces two bf16 matmuls.
Pure e4m3 fails the 2e-2 accuracy gate (rel ~2.6e-2), so a hybrid is used:
on 3 of every 4 m-tiles, d<256 is contracted in one scaled-e4m3 DR matmul
(k*8, Wk*256 - scaling dodges e4m3's denormal floor at |x|<2^-6) and
d>=256 in bf16; every 4th m-tile is all-bf16. Measured rel err ~1.65e-2.
All bf16 W chunks are pre-scaled x2048 to match the fp8 product scale
(8*256); the tanh activation rescales by 1/2048 via its scale operand.

Device pipeline per core (M = 4*4096 = 16384 rows, m-tiles of 512):
    kp[a, m] = sum_d Wk[d, a] * kT[d, m]      (PE: 3 matmuls hybrid / 4 bf16)
    th[a, m] = tanh(kp[a, m]/2048 + qp[a, b] + bias[a])   (ACT, per-partition
                                                           bias, bf16 out)
    score[m] = sum_a v[a] * th[a, m]          (PE, 4 column-tiled 128x32
                                               matmuls run concurrently)
"""

import sys

import ml_dtypes
import numpy as np

for _p in ("/opt/trn_rl_repo",):
    if _p not in sys.path:
        sys.path.append(_p)

import concourse.bass as bass
import concourse.mybir as mybir
import concourse.tile as tile
from concourse import bass_utils

B, T, D, A = 32, 4096, 512, 512
NCORES = 8
BPC = B // NCORES            # batches per core
M = BPC * T                  # rows per core
MT_FREE = 512                # moving free dim per matmul
MT = M // MT_FREE            # m-tiles per core
P = 128
AC = A // P                  # a-chunks
DC = D // P                  # d-chunks
TPB = T // MT_FREE           # m-tiles per batch

SK = 8.0                     # fp8 k scale
SW8 = 256.0                  # fp8 W scale
SCALE = SK * SW8             # psum scale; undone by the tanh activation

F32 = mybir.dt.float32
BF16 = mybir.dt.bfloat16
FP16 = mybir.dt.float16
E4 = mybir.dt.float8e4
E4NP = ml_dtypes.float8_e4m3fn
BF16NP = ml_dtypes.bfloat16
FP16NP = np.float16


def _is_bf16_tile(i):
    # every 4th m-tile all-bf16 keeps the fp8 quantization error at
    # sqrt(3/4) of the pure-hybrid level (rel 1.65e-2 < the 2e-2 gate)
    return i % 4 == 3


def _split_excess_waits(nc, max_waits=1):
    """This walrus build's CoreV3 codegen rejects instructions carrying more
    than one sem wait (setupSyncWait: 'Too many sync wait commands'). Move
    excess waits onto NoOps inserted immediately before the offender — the
    engine executes in order, so sequential waits are equivalent."""
    ctr = 0
    for f in nc.m.functions:
        for blk in f.blocks:
            out = []
            changed = False
            for inst in blk.instructions:
                si = inst.sync_info
                nw = len(si.on_wait) if (si is not None and si.on_wait) else 0
                if nw > max_waits:
                    waits = list(si.on_wait)
                    keep, extra = waits[-max_waits:], waits[:-max_waits]
                    for i in range(0, len(extra), max_waits):
                        nop = mybir.InstNoOp(name=f"I-waitsplit-{ctr}")
                        ctr += 1
                        nop.engine = inst.engine
                        nop.sync_info = mybir.SyncInfo(
                            on_wait=extra[i:i + max_waits], on_update=[])
                        out.append(nop)
                    inst.sync_info = mybir.SyncInfo(
                        on_wait=keep, on_update=list(si.on_update or []))
                    changed = True
                out.append(inst)
            if changed:
                blk.instructions = out
    return ctr


def _build():
    nc = bass.Bass("TRN2", target_bir_lowering=False, debug=False)
    # tile-major layouts: per (partition, m-tile) the payload is contiguous
    kt8 = nc.dram_tensor("kt8", [P, MT, 2, MT_FREE], E4,
                         kind="ExternalInput").ap()
    kthi = nc.dram_tensor("kthi", [P, MT, 2, MT_FREE], FP16,
                          kind="ExternalInput").ap()
    ktlo = nc.dram_tensor("ktlo", [P, MT, 2, MT_FREE], FP16,
                          kind="ExternalInput").ap()
    qT = nc.dram_tensor("qT", [D, BPC], FP16, kind="ExternalInput").ap()
    wq = nc.dram_tensor("wq", [D, A], FP16, kind="ExternalInput").ap()
    wk8 = nc.dram_tensor("wk8", [P, 2, A], E4, kind="ExternalInput").ap()
    wkhi = nc.dram_tensor("wkhi", [P, 2, A], FP16, kind="ExternalInput").ap()
    wklo = nc.dram_tensor("wklo", [P, 2, A], FP16, kind="ExternalInput").ap()
    bias = nc.dram_tensor("bias", [A], F32, kind="ExternalInput").ap()
    vv = nc.dram_tensor("v", [A], FP16, kind="ExternalInput").ap()
    out = nc.dram_tensor("out", [MT, MT_FREE], F32, kind="ExternalOutput").ap()

    GROUP = 4                  # m-tiles per col-tiled v-dot batch
    NB = MT // GROUP

    with tile.TileContext(nc) as tc:
        with tc.tile_pool(name="singles", bufs=1) as singles, \
             tc.tile_pool(name="ktp8", bufs=8) as ktp8, \
             tc.tile_pool(name="ktph", bufs=8) as ktph, \
             tc.tile_pool(name="ktpl", bufs=3) as ktpl, \
             tc.tile_pool(name="thp", bufs=140) as thp, \
             tc.tile_pool(name="scp", bufs=3) as scp, \
             tc.tile_pool(name="psum_kp", bufs=5, space="PSUM") as psum_kp, \
             tc.tile_pool(name="psum_sm", bufs=3, space="PSUM") as psum_sm:

            # PE warm-up: a few matmuls on a zeroed SBUF tile ramp the PE
            # clock while the head DMAs are still in flight. The memset
            # leads the vector queue so warm-up starts immediately.
            warm_sb = singles.tile([P, MT_FREE], FP16, name="warm_sb")
            nc.vector.memset(warm_sb, 0.0)

            # Head DMA routing. A single HWDGE queue moves only ~140GB/s and
            # processes transfers in issue order, so the sync queue is
            # ordered by first-need: wk8 + kt8-0 (the first DR matmul's
            # operands, 2KB/partition) lead, then kthi-0/wkhi for the bf16
            # matmuls that follow ~200ns later. The qp feed rides the gpsimd
            # queue. (Scalar-queue DMAs measurably slow the whole core —
            # avoid.)
            wk8_sb = singles.tile([P, 2, A], E4, name="wk8_sb")
            nc.sync.dma_start(out=wk8_sb, in_=wk8)
            wkhi_sb = None
            wq_sb = singles.tile([P, DC, A], FP16, name="wq_sb")
            nc.gpsimd.dma_start(out=wq_sb, in_=wq.rearrange("(dc p) a -> p dc a", p=P))
            qT_sb = singles.tile([P, DC, BPC], FP16, name="qT_sb")
            nc.gpsimd.dma_start(out=qT_sb, in_=qT.rearrange("(dc p) b -> p dc b", p=P))
            bT_sb = singles.tile([P, AC], F32, name="bT_sb")
            nc.gpsimd.dma_start(out=bT_sb, in_=bias.rearrange("(ac p) -> p ac", p=P))

            kt_tiles = {}

            def get_kt(i):
                if i not in kt_tiles and i < MT:
                    if _is_bf16_tile(i):
                        lo = ktpl.tile([P, 2, MT_FREE], FP16,
                                       name=f"ktlo{i}", tag="ktl")
                        nc.sync.dma_start(out=lo, in_=ktlo[:, i])
                    else:
                        lo = ktp8.tile([P, 2, MT_FREE], E4,
                                       name=f"kt8{i}", tag="kt8")
                        nc.sync.dma_start(out=lo, in_=kt8[:, i])
                    hi = ktph.tile([P, 2, MT_FREE], FP16,
                                   name=f"kthi{i}", tag="kth")
                    nc.sync.dma_start(out=hi, in_=kthi[:, i])
                    kt_tiles[i] = (lo, hi)
                return kt_tiles.get(i)

            get_kt(0)
            wkhi_sb = singles.tile([P, 2, A], FP16, name="wkhi_sb")
            nc.sync.dma_start(out=wkhi_sb, in_=wkhi)
            get_kt(1)
            v_sb = singles.tile([P, AC], FP16, name="v_sb")
            nc.gpsimd.dma_start(out=v_sb, in_=vv.rearrange("(ac p) -> p ac", p=P))
            wklo_sb = singles.tile([P, 2, A], FP16, name="wklo_sb")
            nc.gpsimd.dma_start(out=wklo_sb, in_=wklo)
            qpb_sb = singles.tile([P, AC, BPC], F32, name="qpb_sb")
            for i in range(2, 4):
                get_kt(i)

            # PE warm-up sized to the clock ramp (~3us of continuous busy)
            # so kp starts at full clock right as kt0 lands.
            warm_ps = psum_kp.tile([P, MT_FREE], F32, name="warm_ps", tag="kp")
            for _ in range(7):
                nc.tensor.matmul(warm_ps, lhsT=warm_sb[:, :P], rhs=warm_sb,
                                 start=True, stop=True)

            def vdot_batch(bidx, th_grid):
                # score for GROUP m-tiles in one PSUM bank: strip j holds
                # m-tile j at partition 32j, accumulated over a-chunks.
                # 4 col-tiled matmuls per wave run concurrently (128x32 mode).
                score_ps = psum_sm.tile([P, MT_FREE], F32,
                                        name=f"score_ps{bidx}", tag="sm")
                for ac in range(AC):
                    for j in range(GROUP):
                        nc.tensor.matmul(score_ps[32 * j:32 * j + 1, :],
                                         lhsT=v_sb[:, ac:ac + 1],
                                         rhs=th_grid[j][ac],
                                         start=(ac == 0), stop=(ac == AC - 1),
                                         tile_position=(0, 32 * j))
                sc = scp.tile([P, MT_FREE], F32, name=f"sc{bidx}", tag="sc")
                nc.vector.tensor_copy(sc, score_ps)
                eng = nc.sync if bidx >= NB - 2 else nc.gpsimd
                eng.dma_start(out=out[bidx * GROUP:(bidx + 1) * GROUP, :],
                              in_=sc[0:P:32, :])

            def emit_kp(i, ac):
                lo, hi = kt_tiles[i]
                kp_ps = psum_kp.tile([P, MT_FREE], F32,
                                     name=f"kp{i}_{ac}", tag="kp")
                acs = slice(ac * P, (ac + 1) * P)
                if _is_bf16_tile(i):
                    for dc in range(2):
                        nc.tensor.matmul(kp_ps, lhsT=wklo_sb[:, dc, acs],
                                         rhs=lo[:, dc, :],
                                         start=(dc == 0), stop=False)
                else:
                    # one DoubleRow fp8 matmul contracts d<256 in 512 cycles
                    nc.tensor.matmul(kp_ps, lhsT=wk8_sb[:, :, acs],
                                     rhs=lo,
                                     start=True, stop=False,
                                     perf_mode=mybir.MatmulPerfMode.DoubleRow)
                for dc in range(2):
                    nc.tensor.matmul(kp_ps, lhsT=wkhi_sb[:, dc, acs],
                                     rhs=hi[:, dc, :],
                                     start=False, stop=(dc == 1))
                return kp_ps

            def emit_tanh(i, ac, kp_ps):
                th = thp.tile([P, MT_FREE], FP16, name=f"th{i}_{ac}", tag="th")
                nc.scalar.activation(out=th, in_=kp_ps,
                                     func=mybir.ActivationFunctionType.Tanh,
                                     bias=qpb_sb[:, ac, (i // TPB):(i // TPB) + 1],
                                     scale=1.0 / SCALE)
                return th

            def emit_mtile(i):
                get_kt(i)
                get_kt(i + 4)
                return [emit_tanh(i, ac, emit_kp(i, ac)) for ac in range(AC)]

            # first m-tile's kp matmuls run before qp: they only need wk +
            # kt0, which lead the sync queue. qp (fed via gpsimd) follows;
            # m0's tanh must follow qp (it reads qpb).
            kp0 = [emit_kp(0, ac) for ac in range(AC)]

            # qp[a, b] = sum_d Wq[d, a] q[b, d], then + bias[a]; [a] on partitions
            for ac in range(AC):
                qp_ps = psum_sm.tile([P, BPC], F32, name=f"qp_ps{ac}", tag="sm")
                for dc in range(DC):
                    nc.tensor.matmul(qp_ps,
                                     lhsT=wq_sb[:, dc, ac * P:(ac + 1) * P],
                                     rhs=qT_sb[:, dc, :],
                                     start=(dc == 0), stop=(dc == DC - 1))
                nc.vector.tensor_scalar_add(qpb_sb[:, ac, :], qp_ps,
                                            bT_sb[:, ac:ac + 1])

            th0 = [emit_tanh(0, ac, kp0[ac]) for ac in range(AC)]
            get_kt(4)

            pending = []
            for bi in range(NB):
                if bi == NB - 1:
                    # drain everything before the last batch's kp matmuls so
                    # only one batch's v-dot trails the final kp stream
                    while pending:
                        b0, g0 = pending.pop(0)
                        vdot_batch(b0, g0)
                th_grid = [th0] if bi == 0 else []
                for j in range(1 if bi == 0 else 0, GROUP):
                    th_grid.append(emit_mtile(bi * GROUP + j))
                pending.append((bi, th_grid))
                # drain vdots in groups of 8 batches: one 128x32-mode window
                # per group keeps PE mode switches rare
                if len(pending) >= 9:
                    while len(pending) > 1:
                        b0, g0 = pending.pop(0)
                        vdot_batch(b0, g0)
            for b0, g0 in pending:
                vdot_batch(b0, g0)

    _split_excess_waits(nc)
    return nc


_NC = None


def _get_nc():
    global _NC
    if _NC is None:
        _NC = _build()
    return _NC


def _prep_core(kc):
    """kc: [M, D] f32 for one core -> tile-major kt8/kthi/ktlo arrays."""
    kT = np.ascontiguousarray(kc.T)                      # [D, M]
    # [d, m] -> [p, mt, two, mf] with d = two*128 + p
    def tilemajor(a, dt):
        return np.ascontiguousarray(
            a.reshape(2, P, MT, MT_FREE).transpose(1, 2, 0, 3)).astype(dt)
    return (
        tilemajor(kT[:256] * SK, E4NP),
        tilemajor(kT[256:], FP16NP),
        tilemajor(kT[:256], FP16NP),
    )


def run_sharded(inputs, **run_kwargs):
    q = np.ascontiguousarray(np.asarray(inputs["q"], np.float32))
    k = np.ascontiguousarray(np.asarray(inputs["k"], np.float32))
    W = np.asarray(inputs["W"], np.float32)
    b = np.ascontiguousarray(np.asarray(inputs["b"], np.float32))
    v = np.ascontiguousarray(np.asarray(inputs["v"], np.float32))
    nc = _get_nc()
    wq = np.ascontiguousarray(W[:D])
    wk = np.ascontiguousarray(W[D:])
    wk8 = np.ascontiguousarray(
        (wk[:256] * SW8).reshape(2, P, A).transpose(1, 0, 2)).astype(E4NP)
    wkhi = np.ascontiguousarray(
        (wk[256:] * SCALE).reshape(2, P, A).transpose(1, 0, 2)).astype(FP16NP)
    wklo = np.ascontiguousarray(
        (wk[:256] * SCALE).reshape(2, P, A).transpose(1, 0, 2)).astype(FP16NP)
    in_maps = []
    for c in range(NCORES):
        kc = k[c * BPC:(c + 1) * BPC].reshape(M, D)
        kt8, kthi, ktlo = _prep_core(kc)
        in_maps.append({
            "kt8": kt8, "kthi": kthi, "ktlo": ktlo,
            "qT": np.ascontiguousarray(q[c * BPC:(c + 1) * BPC].T).astype(FP16NP),
            "wq": wq.astype(FP16NP), "wk8": wk8, "wkhi": wkhi, "wklo": wklo,
            "bias": b, "v": v.astype(FP16NP),
        })
    return bass_utils.run_bass_kernel_spmd(nc, in_maps, list(range(NCORES)),
                                           **run_kwargs)


def kernel(q, k, W, b, v):
    res = run_sharded({"q": q, "k": k, "W": W, "b": b, "v": v})
    outs = [res.results[c]["out"].reshape(BPC, T) for c in range(NCORES)]
    return np.concatenate(outs, axis=0)


if __name__ == "__main__":
    rng = np.random.default_rng(0)
    ins = {
        "q": rng.standard_normal((B, D), dtype=np.float32),
        "k": rng.standard_normal((B, T, D), dtype=np.float32),
        "W": (rng.standard_normal((2 * D, A)) * 0.02).astype(np.float32),
        "b": np.zeros((A,), np.float32),
        "v": (rng.standard_normal((A,)) * (2.0 / A) ** 0.5).astype(np.float32),
    }
    got = kernel(**ins)
    Wq, Wk = ins["W"][:D], ins["W"][D:]
    qp = ins["q"] @ Wq
    kp = ins["k"] @ Wk
    ref = np.tanh(qp[:, None, :] + kp + ins["b"]).astype(np.float32) @ ins["v"]
    err = np.abs(got - ref)
    rel = np.linalg.norm(got - ref) / np.linalg.norm(ref)
    print("max abs err:", err.max(), "rel:", rel)
